# revision 21
# baseline (speedup 1.0000x reference)
"""Trainium2 Bass kernel for nn_CorrNet (e3nn-style equivariant MLP + tensor-product head).

Contract: kernel(**inputs) takes the FULL unsharded inputs (as produced by
setup_inputs()) and returns the FULL [N, 1] float32 output.

Strategy (pure data parallel over the atom axis N, 8 NeuronCores):
 - Host: fold every static scalar (1/sqrt(M), 1/sqrt(K), act norms, tp norm,
   output_scale, input_shift) into the weights; eigendecompose the symmetric
   tensor-product forms so the quadratic head becomes
   y = sum_e lam0_e (Q0^T zs)_e^2 + sum_{i,e} lam1_e (Q1^T zv_i)_e^2,
   i.e. pure matmuls + Square activations + a lambda-weighted partition
   reduction (one more matmul with a [128,1] stationary operand).
 - Host: re-layout x into a feature-major [320, N/8] fp16 array per core so
   activations stream through the PE as the moving operand with features on
   partitions.  The 0e block is pre-scaled by C_SILU so the residual update
   is exactly U += silu(ps) with no extra scaling op.
 - Device (per 512-sample tile): 15 small matmuls (fp16 operands, fp32 PSUM),
   Silu/Relu/Square on ScalarE (single 'silu_and_others' table set), gate
   multiplies + vector residual adds on VectorE, scalar residual add on
   GPSIMD, one PSUM->SBUF copy of the [1,512] result row on ScalarE.

Everything is exact algebra up to fp16 storage rounding of activations and
weights; all accumulation is fp32.
"""

import numpy as np

# ---- problem constants (hardcoded per contest contract) ----
M, K, T = 128, 64, 64
N_TOTAL = 131072
N_CORES = 8
NC_SAMP = N_TOTAL // N_CORES  # 16384 samples per core
NT = 512                      # samples per tile
NTILES = NC_SAMP // NT        # 32

C_SILU = 0.5964692111226791
C_RELU = 0.7071067811865186
INV_SQRT_M = float(1.0 / np.sqrt(M))
INV_SQRT_K = float(1.0 / np.sqrt(K))
INV_SQRT_3 = float(1.0 / np.sqrt(3.0))
TP_NORM = float(1.0 / np.sqrt(2.0 * T * T))

# weight-concat column offsets (fp16 [128, NW])
_OFF_LS = (0, 320)
_OFF_LG = (128, 448)
_OFF_BD = (192, 512)
_OFF_LR0 = 640
_OFF_LR1BD = 704
# paired-reduce lhsT columns [128, 2] each
_OFF_RP = 832   # [lam0; 0], [0; lam0]
_OFF_RQ = 834   # [lam1; 0], [0; lam1]
NW = 836
NB = 5  # f32 bias columns: BS1, BS2, BG1dup, BG2dup, BRAdup

_CACHE: dict = {}


def _build_module(n_samp: int, n_tiles: int, act_name: str = "Silu"):
    """Build + compile the Bass/Tile module for one core (n_samp = n_tiles*NT)."""
    from contextlib import ExitStack

    import concourse.bass as bass
    import concourse.tile as tile
    from concourse import bacc, mybir

    f16 = mybir.dt.float16
    f32 = mybir.dt.float32
    AF = mybir.ActivationFunctionType
    AF_SILU = getattr(AF, act_name)

    nc = bacc.Bacc(
        "TRN2",
        target_bir_lowering=False,
        debug=False,
        enable_asserts=False,
        num_devices=N_CORES,
    )
    xh = nc.dram_tensor("xh", [320, n_samp], f16, kind="ExternalInput").ap()
    wcat = nc.dram_tensor("wcat", [128, NW], f16, kind="ExternalInput").ap()
    bcat = nc.dram_tensor("bcat", [128, NB], f32, kind="ExternalInput").ap()
    y = nc.dram_tensor("y", [2, n_samp // 2], f32, kind="ExternalOutput").ap()

    with tile.TileContext(nc) as tc, ExitStack() as ctx:
        wpool = ctx.enter_context(tc.tile_pool(name="w", bufs=1))
        ypool = ctx.enter_context(tc.tile_pool(name="yp", bufs=1))
        inpool = ctx.enter_context(tc.tile_pool(name="inp", bufs=4))
        tmppool = ctx.enter_context(tc.tile_pool(name="tmp", bufs=3))
        ps2 = ctx.enter_context(tc.tile_pool(name="ps2", bufs=2, space="PSUM"))
        psv = ctx.enter_context(tc.tile_pool(name="psv", bufs=3, space="PSUM"))
        ps1 = ctx.enter_context(tc.tile_pool(name="ps1", bufs=1, space="PSUM"))

        W = wpool.tile([128, NW], f16, tag="W")
        nc.sync.dma_start(W[:], wcat[:])
        B = wpool.tile([128, NB], f32, tag="B")
        nc.sync.dma_start(B[:], bcat[:])

        LS = [W[:, _OFF_LS[0]:_OFF_LS[0] + 128], W[:, _OFF_LS[1]:_OFF_LS[1] + 128]]
        LG = [W[:, _OFF_LG[0]:_OFF_LG[0] + 64], W[:, _OFF_LG[1]:_OFF_LG[1] + 64]]
        BD = [W[:, _OFF_BD[0]:_OFF_BD[0] + 128], W[:, _OFF_BD[1]:_OFF_BD[1] + 128]]
        LR0 = W[:, _OFF_LR0:_OFF_LR0 + 64]
        LR1BD = W[:, _OFF_LR1BD:_OFF_LR1BD + 128]
        RP = W[:, _OFF_RP:_OFF_RP + 2]
        RQ = W[:, _OFF_RQ:_OFF_RQ + 2]
        BS = [B[:, 0:1], B[:, 1:2]]
        BG = [B[:, 2:3], B[:, 3:4]]
        BRA = B[:, 4:5]

        Ysb = ypool.tile([2, n_samp // 2], f32, tag="Ysb")

        assert n_tiles % 2 == 0
        for p in range(n_tiles // 2):
            sle = bass.ts(2 * p, NT)
            slo = bass.ts(2 * p + 1, NT)
            U_e = inpool.tile([128, NT], f16, tag="U_e")
            nc.sync.dma_start(U_e[:], xh[0:128, sle])
            U_o = inpool.tile([128, NT], f16, tag="U_o")
            nc.sync.dma_start(U_o[:], xh[0:128, slo])
            # V_X = [comp-X of even tile ; comp-X of odd tile], one 3D-AP DMA each
            xh_t = xh.rearrange("r (t c) -> r t c", c=NT)
            V = []
            for ci in range(3):
                vt = inpool.tile([128, NT], f16, tag=f"V_{ci}")
                r0 = 128 + 64 * ci
                vsrc = xh_t[r0:r0 + 64, 2 * p:2 * p + 2, :].rearrange("r t c -> t r c")
                nc.sync.dma_start(vt[:], vsrc)
                V.append(vt)

            for l in range(2):
                ps_s_e = ps2.tile([128, NT], f32, tag="ps_s")
                nc.tensor.matmul(ps_s_e[:], LS[l], U_e[:], start=True, stop=True)
                ps_s_o = ps2.tile([128, NT], f32, tag="ps_s")
                nc.tensor.matmul(ps_s_o[:], LS[l], U_o[:], start=True, stop=True)
                # gates of both tiles packed into one bank: [g_e ; g_o]
                ps_g = ps1.tile([128, NT], f32, tag="ps_g")
                nc.tensor.matmul(ps_g[0:64, :], LG[l], U_e[:], start=True, stop=True)
                nc.tensor.matmul(
                    ps_g[64:128, :], LG[l], U_o[:],
                    start=True, stop=True, tile_position=(0, 64),
                )
                ps_v = []
                for ci in range(3):
                    pv = psv.tile([128, NT], f32, tag="ps_v")
                    nc.tensor.matmul(pv[:], BD[l], V[ci][:], start=True, stop=True)
                    ps_v.append(pv)

                tmp_s_e = tmppool.tile([128, NT], f16, tag="tmp_s_e")
                nc.scalar.activation(tmp_s_e[:], ps_s_e[:], AF_SILU, bias=BS[l])
                tmp_s_o = tmppool.tile([128, NT], f16, tag="tmp_s_o")
                nc.scalar.activation(tmp_s_o[:], ps_s_o[:], AF_SILU, bias=BS[l])
                g01 = tmppool.tile([128, NT], f16, tag="g01")
                nc.scalar.activation(g01[:], ps_g[:], AF.Relu, bias=BG[l])

                nc.vector.tensor_add(U_e[:], U_e[:], tmp_s_e[:])
                nc.gpsimd.tensor_add(U_o[:], U_o[:], tmp_s_o[:])
                for ci in range(3):
                    tmpv = tmppool.tile([128, NT], f16, tag=f"tmpv_{ci}")
                    nc.vector.tensor_mul(tmpv[:], ps_v[ci][:], g01[:])
                    nc.vector.tensor_add(V[ci][:], V[ci][:], tmpv[:])

            # output head: P = [Q0^T zs_e ; Q0^T zs_o], Q_X = [Q1^T zv_X_e ; Q1^T zv_X_o]
            ps_P = ps2.tile([128, NT], f32, tag="ps_out")
            nc.tensor.matmul(ps_P[0:64, :], LR0, U_e[:], start=True, stop=True)
            nc.tensor.matmul(
                ps_P[64:128, :], LR0, U_o[:],
                start=True, stop=True, tile_position=(0, 64),
            )
            sqP = tmppool.tile([128, NT], f16, tag="sqP")
            nc.scalar.activation(sqP[:], ps_P[:], AF.Square, bias=BRA)
            sqV = []
            for ci in range(3):
                ps_Q = ps2.tile([128, NT], f32, tag="ps_out")
                nc.tensor.matmul(ps_Q[:], LR1BD, V[ci][:], start=True, stop=True)
                sq = tmppool.tile([128, NT], f16, tag=f"sqV_{ci}")
                nc.scalar.activation(sq[:], ps_Q[:], AF.Square)
                sqV.append(sq)

            # lambda-weighted partition reduce -> [2, NT] (partition 0: even, 1: odd)
            ps_y = ps2.tile([2, NT], f32, tag="ps_s")
            nc.tensor.matmul(ps_y[:], RP, sqP[:], start=True, stop=False)
            nc.tensor.matmul(ps_y[:], RQ, sqV[0][:], start=False, stop=False)
            nc.tensor.matmul(ps_y[:], RQ, sqV[1][:], start=False, stop=False)
            nc.tensor.matmul(ps_y[:], RQ, sqV[2][:], start=False, stop=True)
            nc.scalar.copy(Ysb[0:2, bass.ts(p, NT)], ps_y[:])

        nc.sync.dma_start(y[:], Ysb[:])

    nc.compile()
    return nc


def _prep_weights(inputs: dict) -> tuple[np.ndarray, np.ndarray]:
    """Fold all scalars into fp16 stationary operands + f32 bias columns."""
    f64 = np.float64
    w0_1 = np.asarray(inputs["w0_1"], f64)
    b0_1 = np.asarray(inputs["b0_1"], f64)
    w1_1 = np.asarray(inputs["w1_1"], f64)
    w0_2 = np.asarray(inputs["w0_2"], f64)
    b0_2 = np.asarray(inputs["b0_2"], f64)
    w1_2 = np.asarray(inputs["w1_2"], f64)
    w0_o = np.asarray(inputs["w0_o"], f64)
    b0_o = np.asarray(inputs["b0_o"], f64)
    w1_o = np.asarray(inputs["w1_o"], f64)
    w_tp0 = np.asarray(inputs["w_tp0"], f64)
    w_tp1 = np.asarray(inputs["w_tp1"], f64)
    gamma = float(np.asarray(inputs["output_scale"]))

    alpha = 1.0 / C_SILU
    im, ik = INV_SQRT_M, INV_SQRT_K

    W0s = 0.5 * (w_tp0 + w_tp0.T) * TP_NORM / gamma
    W1s = 0.5 * (w_tp1 + w_tp1.T) * INV_SQRT_3 * TP_NORM / gamma
    lam0, Q0 = np.linalg.eigh(W0s)
    lam1, Q1 = np.linalg.eigh(W1s)

    wcat = np.zeros((128, NW), np.float16)
    bcat = np.zeros((128, NB), np.float32)
    for l, (w0, b0, w1) in enumerate(((w0_1, b0_1, w1_1), (w0_2, b0_2, w1_2))):
        wcat[:, _OFF_LS[l]:_OFF_LS[l] + 128] = (alpha * im * w0[:, :128]).astype(np.float16)
        wcat[:, _OFF_LG[l]:_OFF_LG[l] + 64] = (alpha * im / C_RELU * w0[:, 128:]).astype(np.float16)
        bd = ik * w1
        wcat[0:64, _OFF_BD[l]:_OFF_BD[l] + 64] = bd.astype(np.float16)
        wcat[64:128, _OFF_BD[l] + 64:_OFF_BD[l] + 128] = bd.astype(np.float16)
        bcat[:, l] = b0[:128].astype(np.float32)
        gate_b = (b0[128:] / C_RELU).astype(np.float32)
        bcat[0:64, 2 + l] = gate_b
        bcat[64:128, 2 + l] = gate_b
    wcat[:, _OFF_LR0:_OFF_LR0 + 64] = (alpha * im * (w0_o @ Q0)).astype(np.float16)
    lr1 = (ik * (w1_o @ Q1)).astype(np.float16)
    wcat[0:64, _OFF_LR1BD:_OFF_LR1BD + 64] = lr1
    wcat[64:128, _OFF_LR1BD + 64:_OFF_LR1BD + 128] = lr1
    # paired reduce weights: col 0 reduces the even-tile half, col 1 the odd half
    wcat[0:64, _OFF_RP] = lam0.astype(np.float16)
    wcat[64:128, _OFF_RP + 1] = lam0.astype(np.float16)
    wcat[0:64, _OFF_RQ] = lam1.astype(np.float16)
    wcat[64:128, _OFF_RQ + 1] = lam1.astype(np.float16)
    br0 = (Q0.T @ b0_o).astype(np.float32)
    bcat[0:64, 4] = br0
    bcat[64:128, 4] = br0
    return wcat, bcat


def _prep_x(x: np.ndarray, shift: np.ndarray, n_samp: int, n_cores: int = N_CORES) -> list[np.ndarray]:
    """Per-core feature-major fp16 arrays [320, n_samp]."""
    xs_scale = np.float32(C_SILU)
    shift = np.asarray(shift, np.float32)
    out = []
    for c in range(n_cores):
        blk = np.asarray(x[c * n_samp:(c + 1) * n_samp], np.float32) - shift
        arr = np.empty((320, n_samp), np.float16)
        arr[0:128] = (blk[:, :128] * xs_scale).T
        arr[128:192] = blk[:, 128::3].T
        arr[192:256] = blk[:, 129::3].T
        arr[256:320] = blk[:, 130::3].T
        out.append(arr)
    return out


def _get_module():
    if "nc" not in _CACHE:
        _CACHE["nc"] = _build_module(NC_SAMP, NTILES)
    return _CACHE["nc"]


def run(inputs: dict, trace: bool = False):
    """Run on 8 NeuronCores; returns (y [N,1] f32, BassKernelResults)."""
    from concourse import bass_utils
    from concourse.bass_interp import get_hw_module

    nc = _get_module()
    wcat, bcat = _prep_weights(inputs)
    xs = _prep_x(np.asarray(inputs["x"]), np.asarray(inputs["input_shift"]), NC_SAMP)
    in_maps = [{"xh": xs[c], "wcat": wcat, "bcat": bcat} for c in range(N_CORES)]

    old_m = nc.m
    nc.m = get_hw_module(nc.m)
    try:
        res = bass_utils.run_bass_kernel_spmd(
            nc,
            in_maps,
            core_ids=list(range(N_CORES)),
            trace=trace,
        )
    finally:
        nc.m = old_m

    # de-interleave: y dram is [2, NC/2] = (even tiles | odd tiles)
    parts = []
    for c in range(N_CORES):
        yc = res.results[c]["y"]
        arr = np.empty((NTILES, NT), np.float32)
        arr[0::2] = yc[0].reshape(NTILES // 2, NT)
        arr[1::2] = yc[1].reshape(NTILES // 2, NT)
        parts.append(arr.reshape(-1))
    y = np.concatenate(parts)
    return y.astype(np.float32)[:, None], res


def kernel(**inputs) -> np.ndarray:
    y, _ = run(inputs, trace=False)
    return y


# revision 22
# speedup vs baseline: 1.0363x; 1.0363x over previous
"""Trainium2 Bass kernel for nn_CorrNet (e3nn-style equivariant MLP + tensor-product head).

Contract: kernel(**inputs) takes the FULL unsharded inputs (as produced by
setup_inputs()) and returns the FULL [N, 1] float32 output.

Strategy (pure data parallel over the atom axis N, 8 NeuronCores):
 - Host: fold every static scalar (1/sqrt(M), 1/sqrt(K), act norms, tp norm,
   output_scale, input_shift) into the weights; eigendecompose the symmetric
   tensor-product forms so the quadratic head becomes
   y = sum_e lam0_e (Q0^T zs)_e^2 + sum_{i,e} lam1_e (Q1^T zv_i)_e^2,
   i.e. pure matmuls + Square activations + a lambda-weighted partition
   reduction (one more matmul with a [128,1] stationary operand).
 - Host: re-layout x into a feature-major [320, N/8] fp16 array per core so
   activations stream through the PE as the moving operand with features on
   partitions.  The 0e block is pre-scaled by C_SILU so the residual update
   is exactly U += silu(ps) with no extra scaling op.
 - Device (per 512-sample tile): 15 small matmuls (fp16 operands, fp32 PSUM),
   Silu/Relu/Square on ScalarE (single 'silu_and_others' table set), gate
   multiplies + vector residual adds on VectorE, scalar residual add on
   GPSIMD, one PSUM->SBUF copy of the [1,512] result row on ScalarE.

Everything is exact algebra up to fp16 storage rounding of activations and
weights; all accumulation is fp32.
"""

import numpy as np

# ---- problem constants (hardcoded per contest contract) ----
M, K, T = 128, 64, 64
N_TOTAL = 131072
N_CORES = 8
NC_SAMP = N_TOTAL // N_CORES  # 16384 samples per core
NT = 512                      # samples per tile
NTILES = NC_SAMP // NT        # 32

C_SILU = 0.5964692111226791
C_RELU = 0.7071067811865186
INV_SQRT_M = float(1.0 / np.sqrt(M))
INV_SQRT_K = float(1.0 / np.sqrt(K))
INV_SQRT_3 = float(1.0 / np.sqrt(3.0))
TP_NORM = float(1.0 / np.sqrt(2.0 * T * T))

# weight-concat column offsets (fp16 [128, NW])
_OFF_LS = (0, 320)
_OFF_LG = (128, 448)
_OFF_BD = (192, 512)
_OFF_LR0 = 640
_OFF_LR1BD = 704
# paired-reduce lhsT columns [128, 2] each
_OFF_RP = 832   # [lam0; 0], [0; lam0]
_OFF_RQ = 834   # [lam1; 0], [0; lam1]
NW = 836
NB = 5  # f32 bias columns: BS1, BS2, BG1dup, BG2dup, BRAdup

_CACHE: dict = {}


def _build_module(n_samp: int, n_tiles: int, act_name: str = "Silu"):
    """Build + compile the Bass/Tile module for one core (n_samp = n_tiles*NT)."""
    from contextlib import ExitStack

    import concourse.bass as bass
    import concourse.tile as tile
    from concourse import bacc, mybir

    f16 = mybir.dt.float16
    f32 = mybir.dt.float32
    AF = mybir.ActivationFunctionType
    AF_SILU = getattr(AF, act_name)

    nc = bacc.Bacc(
        "TRN2",
        target_bir_lowering=False,
        debug=False,
        enable_asserts=False,
        num_devices=N_CORES,
    )
    xh = nc.dram_tensor("xh", [320, n_samp], f16, kind="ExternalInput").ap()
    wcat = nc.dram_tensor("wcat", [128, NW], f16, kind="ExternalInput").ap()
    bcat = nc.dram_tensor("bcat", [128, NB], f32, kind="ExternalInput").ap()
    y = nc.dram_tensor("y", [2, n_samp // 2], f32, kind="ExternalOutput").ap()

    with tile.TileContext(nc) as tc, ExitStack() as ctx:
        wpool = ctx.enter_context(tc.tile_pool(name="w", bufs=1))
        ypool = ctx.enter_context(tc.tile_pool(name="yp", bufs=1))
        inpool = ctx.enter_context(tc.tile_pool(name="inp", bufs=4))
        tmppool = ctx.enter_context(tc.tile_pool(name="tmp", bufs=3))
        ps2 = ctx.enter_context(tc.tile_pool(name="ps2", bufs=2, space="PSUM"))
        ps3 = ctx.enter_context(tc.tile_pool(name="ps3", bufs=3, space="PSUM"))
        psv = ctx.enter_context(tc.tile_pool(name="psv", bufs=2, space="PSUM"))
        ps1 = ctx.enter_context(tc.tile_pool(name="ps1", bufs=1, space="PSUM"))

        W = wpool.tile([128, NW], f16, tag="W")
        nc.sync.dma_start(W[:], wcat[:])
        B = wpool.tile([128, NB], f32, tag="B")
        nc.sync.dma_start(B[:], bcat[:])

        LS = [W[:, _OFF_LS[0]:_OFF_LS[0] + 128], W[:, _OFF_LS[1]:_OFF_LS[1] + 128]]
        LG = [W[:, _OFF_LG[0]:_OFF_LG[0] + 64], W[:, _OFF_LG[1]:_OFF_LG[1] + 64]]
        BD = [W[:, _OFF_BD[0]:_OFF_BD[0] + 128], W[:, _OFF_BD[1]:_OFF_BD[1] + 128]]
        LR0 = W[:, _OFF_LR0:_OFF_LR0 + 64]
        LR1BD = W[:, _OFF_LR1BD:_OFF_LR1BD + 128]
        RP = W[:, _OFF_RP:_OFF_RP + 2]
        RQ = W[:, _OFF_RQ:_OFF_RQ + 2]
        BS = [B[:, 0:1], B[:, 1:2]]
        BG = [B[:, 2:3], B[:, 3:4]]
        BRA = B[:, 4:5]

        Ysb = ypool.tile([2, n_samp // 2], f32, tag="Ysb")

        assert n_tiles % 2 == 0
        for p in range(n_tiles // 2):
            sle = bass.ts(2 * p, NT)
            slo = bass.ts(2 * p + 1, NT)
            U_e = inpool.tile([128, NT], f16, tag="U_e")
            nc.sync.dma_start(U_e[:], xh[0:128, sle])
            U_o = inpool.tile([128, NT], f16, tag="U_o")
            nc.sync.dma_start(U_o[:], xh[0:128, slo])
            # V_X = [comp-X of even tile ; comp-X of odd tile]
            V = []
            for ci in range(3):
                vt = inpool.tile([128, NT], f16, tag=f"V_{ci}")
                r0 = 128 + 64 * ci
                nc.sync.dma_start(vt[0:64, :], xh[r0:r0 + 64, sle])
                nc.sync.dma_start(vt[64:128, :], xh[r0:r0 + 64, slo])
                V.append(vt)

            for l in range(2):
                ps_s_e = ps3.tile([128, NT], f32, tag="ps_s")
                nc.tensor.matmul(ps_s_e[:], LS[l], U_e[:], start=True, stop=True)
                ps_s_o = ps3.tile([128, NT], f32, tag="ps_s")
                nc.tensor.matmul(ps_s_o[:], LS[l], U_o[:], start=True, stop=True)
                # gates of both tiles packed into one bank: [g_e ; g_o]
                ps_g = ps1.tile([128, NT], f32, tag="ps_g")
                nc.tensor.matmul(ps_g[0:64, :], LG[l], U_e[:], start=True, stop=True)
                nc.tensor.matmul(
                    ps_g[64:128, :], LG[l], U_o[:],
                    start=True, stop=True, tile_position=(0, 64),
                )
                ps_v = []
                for ci in range(3):
                    pv = psv.tile([128, NT], f32, tag="ps_v")
                    nc.tensor.matmul(pv[:], BD[l], V[ci][:], start=True, stop=True)
                    ps_v.append(pv)

                tmp_s_e = tmppool.tile([128, NT], f16, tag="tmp_s_e")
                nc.scalar.activation(tmp_s_e[:], ps_s_e[:], AF_SILU, bias=BS[l])
                tmp_s_o = tmppool.tile([128, NT], f16, tag="tmp_s_o")
                nc.scalar.activation(tmp_s_o[:], ps_s_o[:], AF_SILU, bias=BS[l])
                g01 = tmppool.tile([128, NT], f16, tag="g01")
                nc.scalar.activation(g01[:], ps_g[:], AF.Relu, bias=BG[l])

                nc.vector.tensor_add(U_e[:], U_e[:], tmp_s_e[:])
                nc.gpsimd.tensor_add(U_o[:], U_o[:], tmp_s_o[:])
                for ci in range(3):
                    tmpv = tmppool.tile([128, NT], f16, tag=f"tmpv_{ci}")
                    nc.vector.tensor_mul(tmpv[:], ps_v[ci][:], g01[:])
                    nc.vector.tensor_add(V[ci][:], V[ci][:], tmpv[:])

            # output head: P = [Q0^T zs_e ; Q0^T zs_o], Q_X = [Q1^T zv_X_e ; Q1^T zv_X_o]
            ps_P = ps2.tile([128, NT], f32, tag="ps_out")
            nc.tensor.matmul(ps_P[0:64, :], LR0, U_e[:], start=True, stop=True)
            nc.tensor.matmul(
                ps_P[64:128, :], LR0, U_o[:],
                start=True, stop=True, tile_position=(0, 64),
            )
            sqP = tmppool.tile([128, NT], f16, tag="sqP")
            nc.scalar.activation(sqP[:], ps_P[:], AF.Square, bias=BRA)
            sqV = []
            for ci in range(3):
                ps_Q = ps2.tile([128, NT], f32, tag="ps_out")
                nc.tensor.matmul(ps_Q[:], LR1BD, V[ci][:], start=True, stop=True)
                sq = tmppool.tile([128, NT], f16, tag=f"sqV_{ci}")
                nc.scalar.activation(sq[:], ps_Q[:], AF.Square)
                sqV.append(sq)

            # lambda-weighted partition reduce -> [2, NT] (partition 0: even, 1: odd)
            ps_y = ps3.tile([2, NT], f32, tag="ps_s")
            nc.tensor.matmul(ps_y[:], RP, sqP[:], start=True, stop=False)
            nc.tensor.matmul(ps_y[:], RQ, sqV[0][:], start=False, stop=False)
            nc.tensor.matmul(ps_y[:], RQ, sqV[1][:], start=False, stop=False)
            nc.tensor.matmul(ps_y[:], RQ, sqV[2][:], start=False, stop=True)
            nc.scalar.copy(Ysb[0:2, bass.ts(p, NT)], ps_y[:])

        nc.sync.dma_start(y[:], Ysb[:])

    nc.compile()
    return nc


def _prep_weights(inputs: dict) -> tuple[np.ndarray, np.ndarray]:
    """Fold all scalars into fp16 stationary operands + f32 bias columns."""
    f64 = np.float64
    w0_1 = np.asarray(inputs["w0_1"], f64)
    b0_1 = np.asarray(inputs["b0_1"], f64)
    w1_1 = np.asarray(inputs["w1_1"], f64)
    w0_2 = np.asarray(inputs["w0_2"], f64)
    b0_2 = np.asarray(inputs["b0_2"], f64)
    w1_2 = np.asarray(inputs["w1_2"], f64)
    w0_o = np.asarray(inputs["w0_o"], f64)
    b0_o = np.asarray(inputs["b0_o"], f64)
    w1_o = np.asarray(inputs["w1_o"], f64)
    w_tp0 = np.asarray(inputs["w_tp0"], f64)
    w_tp1 = np.asarray(inputs["w_tp1"], f64)
    gamma = float(np.asarray(inputs["output_scale"]))

    alpha = 1.0 / C_SILU
    im, ik = INV_SQRT_M, INV_SQRT_K

    W0s = 0.5 * (w_tp0 + w_tp0.T) * TP_NORM / gamma
    W1s = 0.5 * (w_tp1 + w_tp1.T) * INV_SQRT_3 * TP_NORM / gamma
    lam0, Q0 = np.linalg.eigh(W0s)
    lam1, Q1 = np.linalg.eigh(W1s)

    wcat = np.zeros((128, NW), np.float16)
    bcat = np.zeros((128, NB), np.float32)
    for l, (w0, b0, w1) in enumerate(((w0_1, b0_1, w1_1), (w0_2, b0_2, w1_2))):
        wcat[:, _OFF_LS[l]:_OFF_LS[l] + 128] = (alpha * im * w0[:, :128]).astype(np.float16)
        wcat[:, _OFF_LG[l]:_OFF_LG[l] + 64] = (alpha * im / C_RELU * w0[:, 128:]).astype(np.float16)
        bd = ik * w1
        wcat[0:64, _OFF_BD[l]:_OFF_BD[l] + 64] = bd.astype(np.float16)
        wcat[64:128, _OFF_BD[l] + 64:_OFF_BD[l] + 128] = bd.astype(np.float16)
        bcat[:, l] = b0[:128].astype(np.float32)
        gate_b = (b0[128:] / C_RELU).astype(np.float32)
        bcat[0:64, 2 + l] = gate_b
        bcat[64:128, 2 + l] = gate_b
    wcat[:, _OFF_LR0:_OFF_LR0 + 64] = (alpha * im * (w0_o @ Q0)).astype(np.float16)
    lr1 = (ik * (w1_o @ Q1)).astype(np.float16)
    wcat[0:64, _OFF_LR1BD:_OFF_LR1BD + 64] = lr1
    wcat[64:128, _OFF_LR1BD + 64:_OFF_LR1BD + 128] = lr1
    # paired reduce weights: col 0 reduces the even-tile half, col 1 the odd half
    wcat[0:64, _OFF_RP] = lam0.astype(np.float16)
    wcat[64:128, _OFF_RP + 1] = lam0.astype(np.float16)
    wcat[0:64, _OFF_RQ] = lam1.astype(np.float16)
    wcat[64:128, _OFF_RQ + 1] = lam1.astype(np.float16)
    br0 = (Q0.T @ b0_o).astype(np.float32)
    bcat[0:64, 4] = br0
    bcat[64:128, 4] = br0
    return wcat, bcat


def _prep_x(x: np.ndarray, shift: np.ndarray, n_samp: int, n_cores: int = N_CORES) -> list[np.ndarray]:
    """Per-core feature-major fp16 arrays [320, n_samp]."""
    xs_scale = np.float32(C_SILU)
    shift = np.asarray(shift, np.float32)
    out = []
    for c in range(n_cores):
        blk = np.asarray(x[c * n_samp:(c + 1) * n_samp], np.float32) - shift
        arr = np.empty((320, n_samp), np.float16)
        arr[0:128] = (blk[:, :128] * xs_scale).T
        arr[128:192] = blk[:, 128::3].T
        arr[192:256] = blk[:, 129::3].T
        arr[256:320] = blk[:, 130::3].T
        out.append(arr)
    return out


def _get_module():
    if "nc" not in _CACHE:
        _CACHE["nc"] = _build_module(NC_SAMP, NTILES)
    return _CACHE["nc"]


def run(inputs: dict, trace: bool = False):
    """Run on 8 NeuronCores; returns (y [N,1] f32, BassKernelResults)."""
    from concourse import bass_utils
    from concourse.bass_interp import get_hw_module

    nc = _get_module()
    wcat, bcat = _prep_weights(inputs)
    xs = _prep_x(np.asarray(inputs["x"]), np.asarray(inputs["input_shift"]), NC_SAMP)
    in_maps = [{"xh": xs[c], "wcat": wcat, "bcat": bcat} for c in range(N_CORES)]

    old_m = nc.m
    nc.m = get_hw_module(nc.m)
    try:
        res = bass_utils.run_bass_kernel_spmd(
            nc,
            in_maps,
            core_ids=list(range(N_CORES)),
            trace=trace,
        )
    finally:
        nc.m = old_m

    # de-interleave: y dram is [2, NC/2] = (even tiles | odd tiles)
    parts = []
    for c in range(N_CORES):
        yc = res.results[c]["y"]
        arr = np.empty((NTILES, NT), np.float32)
        arr[0::2] = yc[0].reshape(NTILES // 2, NT)
        arr[1::2] = yc[1].reshape(NTILES // 2, NT)
        parts.append(arr.reshape(-1))
    y = np.concatenate(parts)
    return y.astype(np.float32)[:, None], res


def kernel(**inputs) -> np.ndarray:
    y, _ = run(inputs, trace=False)
    return y


# revision 24
# speedup vs baseline: 1.2485x; 1.2047x over previous
"""Trainium2 Bass kernel for nn_CorrNet (e3nn-style equivariant MLP + tensor-product head).

Contract: kernel(**inputs) takes the FULL unsharded inputs (as produced by
setup_inputs()) and returns the FULL [N, 1] float32 output.

Strategy (pure data parallel over the atom axis N, 8 NeuronCores):
 - Host: fold every static scalar (1/sqrt(M), 1/sqrt(K), act norms, tp norm,
   output_scale, input_shift) into the weights; eigendecompose the symmetric
   tensor-product forms so the quadratic head becomes
   y = sum_e lam0_e (Q0^T zs)_e^2 + sum_{i,e} lam1_e (Q1^T zv_i)_e^2,
   i.e. pure matmuls + Square activations + a lambda-weighted partition
   reduction (one more matmul with a [128,1] stationary operand).
 - Host: re-layout x into a feature-major [320, N/8] fp16 array per core so
   activations stream through the PE as the moving operand with features on
   partitions.  The 0e block is pre-scaled by C_SILU so the residual update
   is exactly U += silu(ps) with no extra scaling op.
 - Device (per 512-sample tile): 15 small matmuls (fp16 operands, fp32 PSUM),
   Silu/Relu/Square on ScalarE (single 'silu_and_others' table set), gate
   multiplies + vector residual adds on VectorE, scalar residual add on
   GPSIMD, one PSUM->SBUF copy of the [1,512] result row on ScalarE.

Everything is exact algebra up to fp16 storage rounding of activations and
weights; all accumulation is fp32.
"""

import numpy as np

# ---- problem constants (hardcoded per contest contract) ----
M, K, T = 128, 64, 64
N_TOTAL = 131072
N_CORES = 8
NC_SAMP = N_TOTAL // N_CORES  # 16384 samples per core
NT = 512                      # samples per tile
NTILES = NC_SAMP // NT        # 32

C_SILU = 0.5964692111226791
C_RELU = 0.7071067811865186
INV_SQRT_M = float(1.0 / np.sqrt(M))
INV_SQRT_K = float(1.0 / np.sqrt(K))
INV_SQRT_3 = float(1.0 / np.sqrt(3.0))
TP_NORM = float(1.0 / np.sqrt(2.0 * T * T))

# weight-concat column offsets (fp16 [128, NW])
_OFF_LS = (0, 320)
_OFF_LG = (128, 448)
_OFF_BD = (192, 512)
_OFF_LR0 = 640
_OFF_LR1BD = 704
# paired-reduce lhsT columns [128, 2] each
_OFF_RP = 832   # [lam0; 0], [0; lam0]
_OFF_RQ = 834   # [lam1; 0], [0; lam1]
NW = 836
NB = 5  # f32 bias columns: BS1, BS2, BG1dup, BG2dup, BRAdup

_CACHE: dict = {}


def _build_module(n_samp: int, n_tiles: int, act_name: str = "Silu"):
    """Build + compile the Bass/Tile module for one core (n_samp = n_tiles*NT)."""
    from contextlib import ExitStack

    import concourse.bass as bass
    import concourse.tile as tile
    from concourse import bacc, mybir

    f16 = mybir.dt.float16
    f32 = mybir.dt.float32
    AF = mybir.ActivationFunctionType
    AF_SILU = getattr(AF, act_name)

    nc = bacc.Bacc(
        "TRN2",
        target_bir_lowering=False,
        debug=False,
        enable_asserts=False,
        num_devices=N_CORES,
    )
    xh = nc.dram_tensor("xh", [320, n_samp], f16, kind="ExternalInput").ap()
    wcat = nc.dram_tensor("wcat", [128, NW], f16, kind="ExternalInput").ap()
    bcat = nc.dram_tensor("bcat", [128, NB], f32, kind="ExternalInput").ap()
    y = nc.dram_tensor("y", [2, n_samp // 2], f32, kind="ExternalOutput").ap()

    with tile.TileContext(nc) as tc, ExitStack() as ctx:
        wpool = ctx.enter_context(tc.tile_pool(name="w", bufs=1))
        ypool = ctx.enter_context(tc.tile_pool(name="yp", bufs=1))
        inpool = ctx.enter_context(tc.tile_pool(name="inp", bufs=4))
        tmppool = ctx.enter_context(tc.tile_pool(name="tmp", bufs=3))
        ps2 = ctx.enter_context(tc.tile_pool(name="ps2", bufs=2, space="PSUM"))
        psv = ctx.enter_context(tc.tile_pool(name="psv", bufs=3, space="PSUM"))
        ps1 = ctx.enter_context(tc.tile_pool(name="ps1", bufs=1, space="PSUM"))

        W = wpool.tile([128, NW], f16, tag="W")
        nc.sync.dma_start(W[:], wcat[:])
        B = wpool.tile([128, NB], f32, tag="B")
        nc.sync.dma_start(B[:], bcat[:])

        LS = [W[:, _OFF_LS[0]:_OFF_LS[0] + 128], W[:, _OFF_LS[1]:_OFF_LS[1] + 128]]
        LG = [W[:, _OFF_LG[0]:_OFF_LG[0] + 64], W[:, _OFF_LG[1]:_OFF_LG[1] + 64]]
        BD = [W[:, _OFF_BD[0]:_OFF_BD[0] + 128], W[:, _OFF_BD[1]:_OFF_BD[1] + 128]]
        LR0 = W[:, _OFF_LR0:_OFF_LR0 + 64]
        LR1BD = W[:, _OFF_LR1BD:_OFF_LR1BD + 128]
        RP = W[:, _OFF_RP:_OFF_RP + 2]
        RQ = W[:, _OFF_RQ:_OFF_RQ + 2]
        BS = [B[:, 0:1], B[:, 1:2]]
        BG = [B[:, 2:3], B[:, 3:4]]
        BRA = B[:, 4:5]

        Ysb = ypool.tile([2, n_samp // 2], f32, tag="Ysb")

        assert n_tiles % 2 == 0
        for p in range(n_tiles // 2):
            sle = bass.ts(2 * p, NT)
            slo = bass.ts(2 * p + 1, NT)
            U_e = inpool.tile([128, NT], f16, tag="U_e")
            nc.sync.dma_start(U_e[:], xh[0:128, sle])
            U_o = inpool.tile([128, NT], f16, tag="U_o")
            nc.sync.dma_start(U_o[:], xh[0:128, slo])
            # V_X = [comp-X of even tile ; comp-X of odd tile]
            V = []
            for ci in range(3):
                vt = inpool.tile([128, NT], f16, tag=f"V_{ci}")
                r0 = 128 + 64 * ci
                nc.sync.dma_start(vt[0:64, :], xh[r0:r0 + 64, sle])
                nc.sync.dma_start(vt[64:128, :], xh[r0:r0 + 64, slo])
                V.append(vt)

            for l in range(2):
                ps_s_e = ps2.tile([128, NT], f32, tag="ps_s")
                nc.tensor.matmul(ps_s_e[:], LS[l], U_e[:], start=True, stop=True)
                ps_s_o = ps2.tile([128, NT], f32, tag="ps_s")
                nc.tensor.matmul(ps_s_o[:], LS[l], U_o[:], start=True, stop=True)
                # gates of both tiles packed into one bank: [g_e ; g_o]
                ps_g = ps1.tile([128, NT], f32, tag="ps_g")
                nc.tensor.matmul(ps_g[0:64, :], LG[l], U_e[:], start=True, stop=True)
                nc.tensor.matmul(
                    ps_g[64:128, :], LG[l], U_o[:],
                    start=True, stop=True, tile_position=(0, 64),
                )
                ps_v = []
                for ci in range(3):
                    pv = psv.tile([128, NT], f32, tag="ps_v")
                    nc.tensor.matmul(pv[:], BD[l], V[ci][:], start=True, stop=True)
                    ps_v.append(pv)

                tmp_s_e = tmppool.tile([128, NT], f16, tag="tmp_s_e")
                nc.scalar.activation(tmp_s_e[:], ps_s_e[:], AF_SILU, bias=BS[l])
                tmp_s_o = tmppool.tile([128, NT], f16, tag="tmp_s_o")
                nc.scalar.activation(tmp_s_o[:], ps_s_o[:], AF_SILU, bias=BS[l])
                g01 = tmppool.tile([128, NT], f16, tag="g01")
                nc.scalar.activation(g01[:], ps_g[:], AF.Relu, bias=BG[l])

                nc.vector.tensor_add(U_e[:], U_e[:], tmp_s_e[:])
                nc.vector.tensor_add(U_o[:], U_o[:], tmp_s_o[:])
                for ci in range(3):
                    tmpv = tmppool.tile([128, NT], f16, tag=f"tmpv_{ci}")
                    nc.vector.tensor_mul(tmpv[:], ps_v[ci][:], g01[:])
                    nc.vector.tensor_add(V[ci][:], V[ci][:], tmpv[:])

            # output head: P = [Q0^T zs_e ; Q0^T zs_o], Q_X = [Q1^T zv_X_e ; Q1^T zv_X_o]
            ps_P = ps2.tile([128, NT], f32, tag="ps_out")
            nc.tensor.matmul(ps_P[0:64, :], LR0, U_e[:], start=True, stop=True)
            nc.tensor.matmul(
                ps_P[64:128, :], LR0, U_o[:],
                start=True, stop=True, tile_position=(0, 64),
            )
            sqP = tmppool.tile([128, NT], f16, tag="sqP")
            nc.scalar.activation(sqP[:], ps_P[:], AF.Square, bias=BRA)
            sqV = []
            for ci in range(3):
                ps_Q = ps2.tile([128, NT], f32, tag="ps_out")
                nc.tensor.matmul(ps_Q[:], LR1BD, V[ci][:], start=True, stop=True)
                sq = tmppool.tile([128, NT], f16, tag=f"sqV_{ci}")
                nc.scalar.activation(sq[:], ps_Q[:], AF.Square)
                sqV.append(sq)

            # lambda-weighted partition reduce -> [2, NT] (partition 0: even, 1: odd)
            ps_y = ps2.tile([2, NT], f32, tag="ps_s")
            nc.tensor.matmul(ps_y[:], RP, sqP[:], start=True, stop=False)
            nc.tensor.matmul(ps_y[:], RQ, sqV[0][:], start=False, stop=False)
            nc.tensor.matmul(ps_y[:], RQ, sqV[1][:], start=False, stop=False)
            nc.tensor.matmul(ps_y[:], RQ, sqV[2][:], start=False, stop=True)
            nc.scalar.copy(Ysb[0:2, bass.ts(p, NT)], ps_y[:])

        nc.sync.dma_start(y[:], Ysb[:])

    nc.compile()
    return nc


def _prep_weights(inputs: dict) -> tuple[np.ndarray, np.ndarray]:
    """Fold all scalars into fp16 stationary operands + f32 bias columns."""
    f64 = np.float64
    w0_1 = np.asarray(inputs["w0_1"], f64)
    b0_1 = np.asarray(inputs["b0_1"], f64)
    w1_1 = np.asarray(inputs["w1_1"], f64)
    w0_2 = np.asarray(inputs["w0_2"], f64)
    b0_2 = np.asarray(inputs["b0_2"], f64)
    w1_2 = np.asarray(inputs["w1_2"], f64)
    w0_o = np.asarray(inputs["w0_o"], f64)
    b0_o = np.asarray(inputs["b0_o"], f64)
    w1_o = np.asarray(inputs["w1_o"], f64)
    w_tp0 = np.asarray(inputs["w_tp0"], f64)
    w_tp1 = np.asarray(inputs["w_tp1"], f64)
    gamma = float(np.asarray(inputs["output_scale"]))

    alpha = 1.0 / C_SILU
    im, ik = INV_SQRT_M, INV_SQRT_K

    W0s = 0.5 * (w_tp0 + w_tp0.T) * TP_NORM / gamma
    W1s = 0.5 * (w_tp1 + w_tp1.T) * INV_SQRT_3 * TP_NORM / gamma
    lam0, Q0 = np.linalg.eigh(W0s)
    lam1, Q1 = np.linalg.eigh(W1s)

    wcat = np.zeros((128, NW), np.float16)
    bcat = np.zeros((128, NB), np.float32)
    for l, (w0, b0, w1) in enumerate(((w0_1, b0_1, w1_1), (w0_2, b0_2, w1_2))):
        wcat[:, _OFF_LS[l]:_OFF_LS[l] + 128] = (alpha * im * w0[:, :128]).astype(np.float16)
        wcat[:, _OFF_LG[l]:_OFF_LG[l] + 64] = (alpha * im / C_RELU * w0[:, 128:]).astype(np.float16)
        bd = ik * w1
        wcat[0:64, _OFF_BD[l]:_OFF_BD[l] + 64] = bd.astype(np.float16)
        wcat[64:128, _OFF_BD[l] + 64:_OFF_BD[l] + 128] = bd.astype(np.float16)
        bcat[:, l] = b0[:128].astype(np.float32)
        gate_b = (b0[128:] / C_RELU).astype(np.float32)
        bcat[0:64, 2 + l] = gate_b
        bcat[64:128, 2 + l] = gate_b
    wcat[:, _OFF_LR0:_OFF_LR0 + 64] = (alpha * im * (w0_o @ Q0)).astype(np.float16)
    lr1 = (ik * (w1_o @ Q1)).astype(np.float16)
    wcat[0:64, _OFF_LR1BD:_OFF_LR1BD + 64] = lr1
    wcat[64:128, _OFF_LR1BD + 64:_OFF_LR1BD + 128] = lr1
    # paired reduce weights: col 0 reduces the even-tile half, col 1 the odd half
    wcat[0:64, _OFF_RP] = lam0.astype(np.float16)
    wcat[64:128, _OFF_RP + 1] = lam0.astype(np.float16)
    wcat[0:64, _OFF_RQ] = lam1.astype(np.float16)
    wcat[64:128, _OFF_RQ + 1] = lam1.astype(np.float16)
    br0 = (Q0.T @ b0_o).astype(np.float32)
    bcat[0:64, 4] = br0
    bcat[64:128, 4] = br0
    return wcat, bcat


def _prep_x(x: np.ndarray, shift: np.ndarray, n_samp: int, n_cores: int = N_CORES) -> list[np.ndarray]:
    """Per-core feature-major fp16 arrays [320, n_samp]."""
    xs_scale = np.float32(C_SILU)
    shift = np.asarray(shift, np.float32)
    out = []
    for c in range(n_cores):
        blk = np.asarray(x[c * n_samp:(c + 1) * n_samp], np.float32) - shift
        arr = np.empty((320, n_samp), np.float16)
        arr[0:128] = (blk[:, :128] * xs_scale).T
        arr[128:192] = blk[:, 128::3].T
        arr[192:256] = blk[:, 129::3].T
        arr[256:320] = blk[:, 130::3].T
        out.append(arr)
    return out


def _get_module():
    if "nc" not in _CACHE:
        _CACHE["nc"] = _build_module(NC_SAMP, NTILES)
    return _CACHE["nc"]


def run(inputs: dict, trace: bool = False):
    """Run on 8 NeuronCores; returns (y [N,1] f32, BassKernelResults)."""
    from concourse import bass_utils
    from concourse.bass_interp import get_hw_module

    nc = _get_module()
    wcat, bcat = _prep_weights(inputs)
    xs = _prep_x(np.asarray(inputs["x"]), np.asarray(inputs["input_shift"]), NC_SAMP)
    in_maps = [{"xh": xs[c], "wcat": wcat, "bcat": bcat} for c in range(N_CORES)]

    old_m = nc.m
    nc.m = get_hw_module(nc.m)
    try:
        res = bass_utils.run_bass_kernel_spmd(
            nc,
            in_maps,
            core_ids=list(range(N_CORES)),
            trace=trace,
        )
    finally:
        nc.m = old_m

    # de-interleave: y dram is [2, NC/2] = (even tiles | odd tiles)
    parts = []
    for c in range(N_CORES):
        yc = res.results[c]["y"]
        arr = np.empty((NTILES, NT), np.float32)
        arr[0::2] = yc[0].reshape(NTILES // 2, NT)
        arr[1::2] = yc[1].reshape(NTILES // 2, NT)
        parts.append(arr.reshape(-1))
    y = np.concatenate(parts)
    return y.astype(np.float32)[:, None], res


def kernel(**inputs) -> np.ndarray:
    y, _ = run(inputs, trace=False)
    return y
